# revision 14
# baseline (speedup 1.0000x reference)
"""Trainium2 Bass kernel for nn_CausalSE: causal cumulative-average pooling
+ squeeze-excite gating, data-parallel over batch (one NeuronCore per batch
element).

Reference math per batch element (D=512, T=8192, chunk=16, Tc=512):
    avg    = cumsum(x, t) / (t+1)
    pooled = avg[:, 15::16]                          # [D, Tc]
    h      = relu(w1 @ pooled + b1)                  # [64, Tc]
    g      = sigmoid(w2 @ h + b2)                    # [D, Tc]
    out    = repeat(g, 16, t)[:, :T] * x

The kernel is DMA-bound (16 MB in + 16 MB out per core at ~420 GB/s
per-core HBM => ~82 us floor + ~6 us queue startup), and DVE is the
closest compute engine to that floor, so the schedule minimizes both
pipeline-fill time and DVE work:

  - x streams in per-column-block on the sync queue, issued up front;
    the first blocks are small (256/512 cols) so compute starts during
    the DMA rate ramp (packets run ~19 GB/s/engine for the first ~15 us
    before reaching 25).
  - ACT casts each block to fp16; DVE collapses chunks 16->2 with a
    pairwise fp16 add tree (tensor_tensor has the 2x packed-16-bit mode;
    tensor_reduce does not); PE finishes the 2->1 sum for free via PSUM
    accumulation of two j-strided matmuls against w1.
  - The causal prefix scan and the 1/t rescale run on GpSimd (PSUM-read
    scan + tiny tensor_tensor) to keep them off DVE.
  - The gate multiply stays fp32 on DVE only (DVE and GpSimd contend for
    SBUF ports when both run big elementwise ops: measured 2.5x), merged
    over di pairs, lagged one block behind the gate pipeline so DVE
    never stalls on the SE round trip. GpSimd issues the store DMAs.
"""

import sys

for _p in ("/opt/trn_rl_repo",):
    if _p not in sys.path:
        sys.path.insert(0, _p)

import numpy as np

B, D, T = 8, 512, 8192
DH = 64          # bottleneck dim = D // 8
CS = 16          # chunksize
TC = T // CS     # 512 chunks
NCORES = 8
NDT = D // 128   # 4 partition tiles of x / out
# t-block column spans: small blocks lead (pipeline fill during DMA ramp)
# and trail (short tail after the last load)
TBLOCKS = [256, 256, 512] + [1024] * 6 + [512, 256, 256]
assert sum(TBLOCKS) == T
CBMAX = max(TBLOCKS) // CS

_compiled_nc = None


def build_nc():
    import concourse.tile as tile
    from concourse import bacc, mybir

    f32 = mybir.dt.float32
    f16 = mybir.dt.float16
    AF = mybir.ActivationFunctionType
    ALU = mybir.AluOpType

    # Bacc (not plain Bass): its finalize() runs the TRN2 sync-wait
    # legalization (move_matmul_waits_to_ldweights / event-semaphore
    # splitting) that walrus codegen requires.
    nc = bacc.Bacc("TRN2", target_bir_lowering=False)
    x_d = nc.declare_dram_parameter("x", [D, T], f32, isOutput=False)
    w1t_d = nc.declare_dram_parameter("w1t", [D, DH], f16, isOutput=False)
    b1_d = nc.declare_dram_parameter("b1", [DH], f32, isOutput=False)
    w2t_d = nc.declare_dram_parameter("w2t", [DH, D], f16, isOutput=False)
    b2_d = nc.declare_dram_parameter("b2", [D], f32, isOutput=False)
    scale_d = nc.declare_dram_parameter("scale", [DH, TC], f32, isOutput=False)
    out_d = nc.declare_dram_parameter("out", [D, T], f32, isOutput=True)

    with tile.TileContext(nc) as tc:
        with (
            tc.tile_pool(name="xres", bufs=1) as xres,
            tc.tile_pool(name="small", bufs=1) as small,
            tc.tile_pool(name="psum_q", bufs=2, space="PSUM") as psum_q,
            tc.tile_pool(name="psum_g", bufs=4, space="PSUM") as psum_g,
        ):
            # x resident in SBUF: [128, 4, 8192] fp32 = 16 MB, out in place
            xt = xres.tile([128, NDT, T], f32, tag="x")
            # parity-buffered per-block scratch (ACT/PE on block b while
            # DVE runs block b+1)
            xf = [
                small.tile([128, NDT, CBMAX, CS], f16, tag=f"xf{i}", name=f"xf{i}")
                for i in range(2)
            ]
            u3 = [
                small.tile([128, NDT, CBMAX, 2], f16, tag=f"u3{i}", name=f"u3{i}")
                for i in range(2)
            ]
            u1 = small.tile([128, NDT, CBMAX, 8], f16, tag="u1")
            u2 = small.tile([128, NDT, CBMAX, 4], f16, tag="u2")
            gs = small.tile([128, NDT, TC], f32, tag="gs")   # gate
            w1s = small.tile([128, NDT, DH], f16, tag="w1")
            w2s = small.tile([DH, D], f16, tag="w2")
            b1s = small.tile([DH, 1], f32, tag="b1")
            b2s = small.tile([128, NDT], f32, tag="b2")
            scl = small.tile([DH, TC], f32, tag="scl")
            qs = small.tile([DH, TC], f32, tag="qs")    # causal prefix
            h = small.tile([DH, TC], f32, tag="h")
            hf = small.tile([DH, TC], f16, tag="hf")

            # -- replicated weights / constants on the GpSimd queue, so the
            # sync queue's 16 MB x prefetch starts at first byte and ACT is
            # free for the fp16 casts --
            for ki in range(NDT):
                nc.gpsimd.dma_start(
                    w1s[:, ki, :], w1t_d[ki * 128:(ki + 1) * 128, :]
                )
                nc.gpsimd.dma_start(
                    b2s[:, ki:ki + 1],
                    b2_d[ki * 128:(ki + 1) * 128].unsqueeze(1),
                )
            nc.gpsimd.dma_start(w2s[:], w2t_d[:])
            nc.gpsimd.dma_start(b1s[:], b1_d[:].unsqueeze(1))
            nc.gpsimd.dma_start(scl[:], scale_d[:])

            # All loads issue up front on the sync queue: nothing may sit
            # between them, or an in-order store wait would stall prefetch.
            # One dma_start per block (all di via a DRAM-side rearrange):
            # ring-writes cost ~0.6 us each, and per-di issue would let the
            # load queue run dry during the first ~10 us.
            t0s = [sum(TBLOCKS[:b]) for b in range(len(TBLOCKS))]
            for b, TB in enumerate(TBLOCKS):
                t0 = t0s[b]
                nc.sync.dma_start(
                    xt[:, :, t0:t0 + TB],
                    x_d[:, t0:t0 + TB].rearrange("(d p) t -> p d t", d=NDT),
                )

            # Causal pipeline: gate for chunk c needs only x[:, :16(c+1)].
            # The gate-multiply + store for block b-1 issue while block b's
            # gate computes, so DVE never waits on the SE round trip.
            NB = len(TBLOCKS)

            def cast_block(b):
                # fp16 cast of the whole block (all di) on ACT. Issued one
                # block AHEAD of the block's SE ops so ACT's in-order queue
                # never serializes the cast behind the previous sigmoid
                # (which would put the whole SE round trip on DVE's path).
                TB = TBLOCKS[b]
                t0 = t0s[b]
                nc.scalar.activation(
                    xf[b % 2][:, :, :TB // CS, :],
                    xt[:, :, t0:t0 + TB].rearrange(
                        "p d (c j) -> p d c j", j=CS
                    ),
                    AF.Copy,
                )

            # Software pipeline, one iteration per block. In-order engine
            # queues make any cross-block producer/consumer interleaving a
            # serial loop, so the only cross-block chain allowed is the
            # scan's carry (scan(b) -> scan(b+1)), covered by DVE's own
            # work: PE runs qmm(i) BEFORE gmm(i-1) so the gate matmul never
            # blocks the next scan's input, ACT runs cast(i+1) before
            # sig(i-1)/relu(i), and the gate-multiply lags TWO blocks so
            # sigmoid latency never stalls DVE.
            cast_block(0)
            qps = {}
            for i in range(NB + 2):
                if i + 1 < NB:
                    cast_block(i + 1)
                if i < NB:
                    TB = TBLOCKS[i]
                    t0 = t0s[i]
                    CB = TB // CS
                    xfb = xf[i % 2]
                    u3b = u3[i % 2]
                    # chunk sums 16->2 via pairwise fp16 adds (2x DVE mode)
                    with nc.allow_low_precision(reason="fp16 chunk sums"):
                        nc.vector.tensor_tensor(
                            u1[:, :, :CB, :],
                            xfb[:, :, :CB, 0:8], xfb[:, :, :CB, 8:16],
                            op=ALU.add,
                        )
                        nc.vector.tensor_tensor(
                            u2[:, :, :CB, :],
                            u1[:, :, :CB, 0:4], u1[:, :, :CB, 4:8],
                            op=ALU.add,
                        )
                        nc.vector.tensor_tensor(
                            u3b[:, :, :CB, :],
                            u2[:, :, :CB, 0:2], u2[:, :, :CB, 2:4],
                            op=ALU.add,
                        )
                    # q = w1 @ s: PSUM accumulation finishes the 2->1 sum
                    qp = psum_q.tile([DH, CB], f32, tag="q", name="qp")
                    qps[i] = qp
                    for ki in range(NDT):
                        for j in range(2):
                            nc.tensor.matmul(
                                qp[:],
                                w1s[:, ki, :],
                                u3b[:, ki, :CB, j],
                                start=(ki == 0 and j == 0),
                                stop=(ki == NDT - 1 and j == 1),
                            )
                if i >= 1 and i - 1 < NB:
                    # gate matmul + sigmoid for the previous block (on PE
                    # strictly AFTER this block's q matmuls)
                    bg = i - 1
                    c0g = t0s[bg] // CS
                    CBg = TBLOCKS[bg] // CS
                    for di in range(NDT):
                        gp = psum_g.tile([128, CBMAX], f32, tag="g", name="gp")
                        nc.tensor.matmul(
                            gp[:, :CBg],
                            w2s[:, di * 128:(di + 1) * 128],
                            hf[:, c0g:c0g + CBg],
                            start=True,
                            stop=True,
                        )
                        nc.scalar.activation(
                            gs[:, di, c0g:c0g + CBg], gp[:, :CBg], AF.Sigmoid,
                            bias=b2s[:, di:di + 1],
                        )
                def lagged_mult():
                    bm = i - 2
                    TBm = TBLOCKS[bm]
                    t0m = t0s[bm]
                    c0m = t0m // CS
                    CBm = TBm // CS
                    # gate-multiply two blocks back, in place on DVE only
                    # (GpSimd elementwise would contend for the shared SBUF
                    # ports); GpSimd issues the stores
                    for dp in range(2):
                        xv = xt[:, 2 * dp:2 * dp + 2, t0m:t0m + TBm].rearrange(
                            "p d (c j) -> p d c j", j=CS
                        )
                        gv = (
                            gs[:, 2 * dp:2 * dp + 2, c0m:c0m + CBm]
                            .unsqueeze(3)
                            .broadcast_to([128, 2, CBm, CS])
                        )
                        nc.vector.tensor_tensor(xv, xv, gv, op=ALU.mult)
                        nc.gpsimd.dma_start(
                            out_d[2 * dp * 128:(2 * dp + 2) * 128,
                                  t0m:t0m + TBm]
                            .rearrange("(d p) t -> p d t", d=2),
                            xt[:, 2 * dp:2 * dp + 2, t0m:t0m + TBm],
                        )

                def se_chain():
                    c0 = t0s[i] // CS
                    CB = TBLOCKS[i] // CS
                    # causal prefix + 1/t rescale + relu for this block
                    nc.vector.tensor_tensor_scan(
                        qs[:, c0:c0 + CB],
                        qps[i][:],
                        scl[:, c0:c0 + CB],
                        0.0 if i == 0 else qs[:, c0 - 1:c0],
                        op0=ALU.add,
                        op1=ALU.bypass,
                    )
                    nc.vector.tensor_mul(
                        h[:, c0:c0 + CB], qs[:, c0:c0 + CB],
                        scl[:, c0:c0 + CB],
                    )
                    nc.scalar.activation(
                        hf[:, c0:c0 + CB], h[:, c0:c0 + CB], AF.Relu,
                        bias=b1s[:, :1],
                    )

                # Mid-pipeline the lagged mult goes FIRST on DVE so the scan
                # never stalls waiting for its q matmul. In the last two
                # iterations the order flips: the final SE chains are the
                # tail's critical path, while the trailing mults overlap the
                # store-backlog drain.
                if i >= NB - 2:
                    if i < NB:
                        se_chain()
                    if i >= 2:
                        lagged_mult()
                else:
                    if i >= 2:
                        lagged_mult()
                    if i < NB:
                        se_chain()
    # run_bass_via_pjrt serializes nc.m as-is; Bacc defers register
    # allocation and TRN2 sync-wait legalization to finalize(), so it must
    # run here or walrus rejects the BIR.
    nc.finalize()
    return nc


def _host_inputs(x, w1, b1, w2, b2, chunksize):
    x = np.ascontiguousarray(np.asarray(x, dtype=np.float32))
    w1 = np.asarray(w1, dtype=np.float32)
    b1 = np.ascontiguousarray(np.asarray(b1, dtype=np.float32))
    w2 = np.asarray(w2, dtype=np.float32)
    b2 = np.ascontiguousarray(np.asarray(b2, dtype=np.float32))
    cs = int(chunksize)
    assert cs == CS and x.shape == (B, D, T), (cs, x.shape)
    w1t = np.ascontiguousarray(w1.T.astype(np.float16))   # [D, DH] fp16
    w2t = np.ascontiguousarray(w2.T.astype(np.float16))   # [DH, D] fp16
    scale = np.broadcast_to(
        1.0 / (CS * np.arange(1, TC + 1, dtype=np.float32)), (DH, TC)
    )
    scale = np.ascontiguousarray(scale)
    shared = dict(w1t=w1t, b1=b1, w2t=w2t, b2=b2, scale=scale)
    return x, shared


def kernel(x, w1, b1, w2, b2, chunksize):
    global _compiled_nc
    from concourse.bass_utils import run_bass_kernel_spmd

    x, shared = _host_inputs(x, w1, b1, w2, b2, chunksize)
    if _compiled_nc is None:
        _compiled_nc = build_nc()
    in_maps = [
        {"x": np.ascontiguousarray(x[i]), **shared} for i in range(NCORES)
    ]
    res = run_bass_kernel_spmd(_compiled_nc, in_maps, list(range(NCORES)))
    out = np.stack([res.results[i]["out"] for i in range(NCORES)], axis=0)
    return out


# revision 16
# speedup vs baseline: 1.1105x; 1.1105x over previous
"""Trainium2 Bass kernel for nn_CausalSE: causal cumulative-average pooling
+ squeeze-excite gating, data-parallel over batch (one NeuronCore per batch
element).

Reference math per batch element (D=512, T=8192, chunk=16, Tc=512):
    avg    = cumsum(x, t) / (t+1)
    pooled = avg[:, 15::16]                          # [D, Tc]
    h      = relu(w1 @ pooled + b1)                  # [64, Tc]
    g      = sigmoid(w2 @ h + b2)                    # [D, Tc]
    out    = repeat(g, 16, t)[:, :T] * x

The kernel is DMA-bound (16 MB in + 16 MB out per core at ~420 GB/s
per-core HBM => ~82 us floor + ~6 us queue startup), and DVE is the
closest compute engine to that floor, so the schedule minimizes both
pipeline-fill time and DVE work:

  - x streams in per-column-block on the sync queue, issued up front;
    the first blocks are small (256/512 cols) so compute starts during
    the DMA rate ramp (packets run ~19 GB/s/engine for the first ~15 us
    before reaching 25).
  - ACT casts each block to fp16; DVE collapses chunks 16->2 with a
    pairwise fp16 add tree (tensor_tensor has the 2x packed-16-bit mode;
    tensor_reduce does not); PE finishes the 2->1 sum for free via PSUM
    accumulation of two j-strided matmuls against w1.
  - The causal prefix scan and the 1/t rescale run on GpSimd (PSUM-read
    scan + tiny tensor_tensor) to keep them off DVE.
  - The gate multiply stays fp32 on DVE only (DVE and GpSimd contend for
    SBUF ports when both run big elementwise ops: measured 2.5x), merged
    over di pairs, lagged one block behind the gate pipeline so DVE
    never stalls on the SE round trip. GpSimd issues the store DMAs.
"""

import sys

for _p in ("/opt/trn_rl_repo",):
    if _p not in sys.path:
        sys.path.insert(0, _p)

import numpy as np

B, D, T = 8, 512, 8192
DH = 64          # bottleneck dim = D // 8
CS = 16          # chunksize
TC = T // CS     # 512 chunks
NCORES = 8
NDT = D // 128   # 4 partition tiles of x / out
# t-block column spans: small blocks lead (pipeline fill during DMA ramp)
# and trail (short tail after the last load)
TBLOCKS = [256, 256, 512] + [1024] * 6 + [512, 256, 256]
assert sum(TBLOCKS) == T
CBMAX = max(TBLOCKS) // CS

_compiled_nc = None


def build_nc():
    import concourse.tile as tile
    from concourse import bacc, mybir

    f32 = mybir.dt.float32
    f16 = mybir.dt.float16
    AF = mybir.ActivationFunctionType
    ALU = mybir.AluOpType

    # Bacc (not plain Bass): its finalize() runs the TRN2 sync-wait
    # legalization (move_matmul_waits_to_ldweights / event-semaphore
    # splitting) that walrus codegen requires.
    nc = bacc.Bacc("TRN2", target_bir_lowering=False)
    x_d = nc.declare_dram_parameter("x", [D, T], f32, isOutput=False)
    w1t_d = nc.declare_dram_parameter("w1t", [D, DH], f16, isOutput=False)
    b1_d = nc.declare_dram_parameter("b1", [DH], f32, isOutput=False)
    w2t_d = nc.declare_dram_parameter("w2t", [DH, D], f16, isOutput=False)
    b2_d = nc.declare_dram_parameter("b2", [D], f32, isOutput=False)
    scale_d = nc.declare_dram_parameter("scale", [DH, TC], f32, isOutput=False)
    out_d = nc.declare_dram_parameter("out", [D, T], f32, isOutput=True)

    with tile.TileContext(nc) as tc:
        with (
            tc.tile_pool(name="xres", bufs=1) as xres,
            tc.tile_pool(name="small", bufs=1) as small,
            tc.tile_pool(name="psum_q", bufs=2, space="PSUM") as psum_q,
            tc.tile_pool(name="psum_g", bufs=4, space="PSUM") as psum_g,
        ):
            # x resident in SBUF: [128, 4, 8192] fp32 = 16 MB, out in place
            xt = xres.tile([128, NDT, T], f32, tag="x")
            # parity-buffered per-block scratch (ACT/PE on block b while
            # DVE runs block b+1)
            xf = [
                small.tile([128, NDT, CBMAX, CS], f16, tag=f"xf{i}", name=f"xf{i}")
                for i in range(2)
            ]
            u3 = [
                small.tile([128, NDT, CBMAX, 2], f16, tag=f"u3{i}", name=f"u3{i}")
                for i in range(2)
            ]
            u1 = small.tile([128, NDT, CBMAX, 8], f16, tag="u1")
            u2 = small.tile([128, NDT, CBMAX, 4], f16, tag="u2")
            gs = small.tile([128, NDT, TC], f32, tag="gs")   # gate
            w1s = small.tile([128, NDT, DH], f16, tag="w1")
            w2s = small.tile([DH, D], f16, tag="w2")
            b1s = small.tile([DH, 1], f32, tag="b1")
            b2s = small.tile([128, NDT], f32, tag="b2")
            scl = small.tile([DH, TC], f32, tag="scl")
            qs = small.tile([DH, TC], f32, tag="qs")    # causal prefix
            h = small.tile([DH, TC], f32, tag="h")
            hf = small.tile([DH, TC], f16, tag="hf")

            # -- replicated weights / constants on the GpSimd queue, so the
            # sync queue's 16 MB x prefetch starts at first byte and ACT is
            # free for the fp16 casts --
            for ki in range(NDT):
                nc.gpsimd.dma_start(
                    w1s[:, ki, :], w1t_d[ki * 128:(ki + 1) * 128, :]
                )
                nc.gpsimd.dma_start(
                    b2s[:, ki:ki + 1],
                    b2_d[ki * 128:(ki + 1) * 128].unsqueeze(1),
                )
            nc.gpsimd.dma_start(w2s[:], w2t_d[:])
            nc.gpsimd.dma_start(b1s[:], b1_d[:].unsqueeze(1))
            nc.gpsimd.dma_start(scl[:], scale_d[:])

            # All loads issue up front on the sync queue: nothing may sit
            # between them, or an in-order store wait would stall prefetch.
            # All loads issue up front on the sync queue: nothing may sit
            # between them, or an in-order store wait would stall prefetch.
            # Per-di dma_starts: a merged all-di AP fragments the DMA
            # packets (2.6 KB vs 3.5 KB) and loses ~15% bandwidth.
            t0s = [sum(TBLOCKS[:b]) for b in range(len(TBLOCKS))]
            for b, TB in enumerate(TBLOCKS):
                t0 = t0s[b]
                for di in range(NDT):
                    nc.sync.dma_start(
                        xt[:, di, t0:t0 + TB],
                        x_d[di * 128:(di + 1) * 128, t0:t0 + TB],
                    )

            # Causal pipeline: gate for chunk c needs only x[:, :16(c+1)].
            # The gate-multiply + store for block b-1 issue while block b's
            # gate computes, so DVE never waits on the SE round trip.
            NB = len(TBLOCKS)

            def cast_block(b):
                # fp16 cast of the whole block (all di) on ACT. Issued one
                # block AHEAD of the block's SE ops so ACT's in-order queue
                # never serializes the cast behind the previous sigmoid
                # (which would put the whole SE round trip on DVE's path).
                TB = TBLOCKS[b]
                t0 = t0s[b]
                nc.scalar.activation(
                    xf[b % 2][:, :, :TB // CS, :],
                    xt[:, :, t0:t0 + TB].rearrange(
                        "p d (c j) -> p d c j", j=CS
                    ),
                    AF.Copy,
                )

            # Software pipeline, one iteration per block. In-order engine
            # queues make any cross-block producer/consumer interleaving a
            # serial loop, so the only cross-block chain allowed is the
            # scan's carry (scan(b) -> scan(b+1)), covered by DVE's own
            # work: PE runs qmm(i) BEFORE gmm(i-1) so the gate matmul never
            # blocks the next scan's input, ACT runs cast(i+1) before
            # sig(i-1)/relu(i), and the gate-multiply lags TWO blocks so
            # sigmoid latency never stalls DVE.
            cast_block(0)
            qps = {}
            for i in range(NB + 2):
                if i + 1 < NB:
                    cast_block(i + 1)
                if i < NB:
                    TB = TBLOCKS[i]
                    t0 = t0s[i]
                    CB = TB // CS
                    xfb = xf[i % 2]
                    u3b = u3[i % 2]
                    # chunk sums 16->2 via pairwise fp16 adds (2x DVE mode)
                    with nc.allow_low_precision(reason="fp16 chunk sums"):
                        nc.vector.tensor_tensor(
                            u1[:, :, :CB, :],
                            xfb[:, :, :CB, 0:8], xfb[:, :, :CB, 8:16],
                            op=ALU.add,
                        )
                        nc.vector.tensor_tensor(
                            u2[:, :, :CB, :],
                            u1[:, :, :CB, 0:4], u1[:, :, :CB, 4:8],
                            op=ALU.add,
                        )
                        nc.vector.tensor_tensor(
                            u3b[:, :, :CB, :],
                            u2[:, :, :CB, 0:2], u2[:, :, :CB, 2:4],
                            op=ALU.add,
                        )
                    # q = w1 @ s: PSUM accumulation finishes the 2->1 sum
                    qp = psum_q.tile([DH, CB], f32, tag="q", name="qp")
                    qps[i] = qp
                    for ki in range(NDT):
                        for j in range(2):
                            nc.tensor.matmul(
                                qp[:],
                                w1s[:, ki, :],
                                u3b[:, ki, :CB, j],
                                start=(ki == 0 and j == 0),
                                stop=(ki == NDT - 1 and j == 1),
                            )
                if i >= 1 and i - 1 < NB:
                    # gate matmul + sigmoid for the previous block (on PE
                    # strictly AFTER this block's q matmuls)
                    bg = i - 1
                    c0g = t0s[bg] // CS
                    CBg = TBLOCKS[bg] // CS
                    for di in range(NDT):
                        gp = psum_g.tile([128, CBMAX], f32, tag="g", name="gp")
                        nc.tensor.matmul(
                            gp[:, :CBg],
                            w2s[:, di * 128:(di + 1) * 128],
                            hf[:, c0g:c0g + CBg],
                            start=True,
                            stop=True,
                        )
                        nc.scalar.activation(
                            gs[:, di, c0g:c0g + CBg], gp[:, :CBg], AF.Sigmoid,
                            bias=b2s[:, di:di + 1],
                        )
                def lagged_mult():
                    bm = i - 2
                    TBm = TBLOCKS[bm]
                    t0m = t0s[bm]
                    c0m = t0m // CS
                    CBm = TBm // CS
                    # gate-multiply two blocks back, in place on DVE only
                    # (GpSimd elementwise would contend for the shared SBUF
                    # ports); GpSimd issues the stores
                    for dp in range(2):
                        xv = xt[:, 2 * dp:2 * dp + 2, t0m:t0m + TBm].rearrange(
                            "p d (c j) -> p d c j", j=CS
                        )
                        gv = (
                            gs[:, 2 * dp:2 * dp + 2, c0m:c0m + CBm]
                            .unsqueeze(3)
                            .broadcast_to([128, 2, CBm, CS])
                        )
                        nc.vector.tensor_tensor(xv, xv, gv, op=ALU.mult)
                        # stores ride the SYNC queue BEHIND all the loads:
                        # the FIFO gives loads strict priority (they finish
                        # ~25 us earlier, so the tail SE chains hide behind
                        # the store-backlog drain), and the final queue
                        # drain is a fast HWDGE drain instead of SWDGE's.
                        nc.sync.dma_start(
                            out_d[2 * dp * 128:(2 * dp + 2) * 128,
                                  t0m:t0m + TBm]
                            .rearrange("(d p) t -> p d t", d=2),
                            xt[:, 2 * dp:2 * dp + 2, t0m:t0m + TBm],
                        )

                def se_chain():
                    c0 = t0s[i] // CS
                    CB = TBLOCKS[i] // CS
                    # causal prefix + 1/t rescale + relu for this block
                    nc.vector.tensor_tensor_scan(
                        qs[:, c0:c0 + CB],
                        qps[i][:],
                        scl[:, c0:c0 + CB],
                        0.0 if i == 0 else qs[:, c0 - 1:c0],
                        op0=ALU.add,
                        op1=ALU.bypass,
                    )
                    nc.vector.tensor_mul(
                        h[:, c0:c0 + CB], qs[:, c0:c0 + CB],
                        scl[:, c0:c0 + CB],
                    )
                    nc.scalar.activation(
                        hf[:, c0:c0 + CB], h[:, c0:c0 + CB], AF.Relu,
                        bias=b1s[:, :1],
                    )

                # Mid-pipeline the lagged mult goes FIRST on DVE so the scan
                # never stalls waiting for its q matmul. In the last two
                # iterations the order flips: the final SE chains are the
                # tail's critical path, while the trailing mults overlap the
                # store-backlog drain.
                if i >= NB - 2:
                    if i < NB:
                        se_chain()
                    if i >= 2:
                        lagged_mult()
                else:
                    if i >= 2:
                        lagged_mult()
                    if i < NB:
                        se_chain()
    # run_bass_via_pjrt serializes nc.m as-is; Bacc defers register
    # allocation and TRN2 sync-wait legalization to finalize(), so it must
    # run here or walrus rejects the BIR.
    nc.finalize()
    return nc


def _host_inputs(x, w1, b1, w2, b2, chunksize):
    x = np.ascontiguousarray(np.asarray(x, dtype=np.float32))
    w1 = np.asarray(w1, dtype=np.float32)
    b1 = np.ascontiguousarray(np.asarray(b1, dtype=np.float32))
    w2 = np.asarray(w2, dtype=np.float32)
    b2 = np.ascontiguousarray(np.asarray(b2, dtype=np.float32))
    cs = int(chunksize)
    assert cs == CS and x.shape == (B, D, T), (cs, x.shape)
    w1t = np.ascontiguousarray(w1.T.astype(np.float16))   # [D, DH] fp16
    w2t = np.ascontiguousarray(w2.T.astype(np.float16))   # [DH, D] fp16
    scale = np.broadcast_to(
        1.0 / (CS * np.arange(1, TC + 1, dtype=np.float32)), (DH, TC)
    )
    scale = np.ascontiguousarray(scale)
    shared = dict(w1t=w1t, b1=b1, w2t=w2t, b2=b2, scale=scale)
    return x, shared


def kernel(x, w1, b1, w2, b2, chunksize):
    global _compiled_nc
    from concourse.bass_utils import run_bass_kernel_spmd

    x, shared = _host_inputs(x, w1, b1, w2, b2, chunksize)
    if _compiled_nc is None:
        _compiled_nc = build_nc()
    in_maps = [
        {"x": np.ascontiguousarray(x[i]), **shared} for i in range(NCORES)
    ]
    res = run_bass_kernel_spmd(_compiled_nc, in_maps, list(range(NCORES)))
    out = np.stack([res.results[i]["out"] for i in range(NCORES)], axis=0)
    return out
